# revision 8
# baseline (speedup 1.0000x reference)
"""AUGRU (attention-modulated GRU) Trainium2 kernel.

Problem: B=2048, T=200, D=H=64.  Data-parallel over 8 NeuronCores
(256 batch rows per core); the T=200 recurrence runs on-device.

Math per step (reference):
    gates = sigmoid([x, h] @ Wg + bg);  r, u = split(gates)
    c     = tanh([x, r*h] @ Wc + bc)
    u_hat = (1 - att) * u
    h'    = u_hat * h + (1 - u_hat) * c
    out_t = valid ? h' : 0;  h = valid ? h' : h      (valid = t < seq_len)

Device-side reformulation (feature-major layout [feat, batch]):
  * The valid-mask is NOT applied on device: invalid steps only corrupt
    outputs at t >= seq_len, which the host zeroes afterwards.
  * State kept as h~ = h + 1 so that affine corrections fold into
    scalar_tensor_tensor ops; weight/bias corrections are folded on host.
  * tanh(z) = 2*sigmoid(2z) - 1: candidate weights pre-scaled by 2 on host
    so only Sigmoid is ever evaluated (one ACT table set).
  * s  = sigmoid(2*zc);  c - h = 2s - h~;  h~' = 2s - u_hat*(2s - h~).
  * (1-att) is broadcast across the 64 feature partitions with a K=1 matmul.
Matmuls run in float32r (full-rate fp32 PE mode, ~1e-5 rel precision).
"""

import numpy as np

B, T, D, H = 2048, 200, 64, 64
NCORES = 8
BL = B // NCORES  # 256

_CACHE = {}


def _build():
    """Build + compile the per-core Bass module (same SPMD program on all cores)."""
    if "nc" in _CACHE:
        return _CACHE["nc"]

    from contextlib import ExitStack
    import concourse.tile as tile
    from concourse import bacc, mybir

    f32 = mybir.dt.float32
    f32r = mybir.dt.float32r
    ALU = mybir.AluOpType
    AF = mybir.ActivationFunctionType

    nc = bacc.Bacc("TRN2", target_bir_lowering=False, debug=False,
                   enable_asserts=False, num_devices=NCORES)

    X = nc.dram_tensor("x", [T, D, BL], f32, kind="ExternalInput").ap()
    Qd = nc.dram_tensor("q", [T, BL], f32, kind="ExternalInput").ap()
    W1 = nc.dram_tensor("w1", [D + H, 2 * H], f32, kind="ExternalInput").ap()
    ADJB = nc.dram_tensor("adjb", [1, 2 * H], f32, kind="ExternalInput").ap()
    W2X = nc.dram_tensor("w2x", [D, H], f32, kind="ExternalInput").ap()
    W2HE = nc.dram_tensor("w2he", [H + 1, H], f32, kind="ExternalInput").ap()
    ONES = nc.dram_tensor("ones", [H, BL], f32, kind="ExternalInput").ap()
    OUT = nc.dram_tensor("out", [T, H, BL], f32, kind="ExternalOutput").ap()

    def R(ap):
        return ap.bitcast(f32r)

    with tile.TileContext(nc) as tc:
        with ExitStack() as ctx:
            consts = ctx.enter_context(tc.tile_pool(name="consts", bufs=1))
            state = ctx.enter_context(tc.tile_pool(name="state", bufs=1))
            gpool = ctx.enter_context(tc.tile_pool(name="gates", bufs=3))
            tpool = ctx.enter_context(tc.tile_pool(name="tmp", bufs=3))
            qpool = ctx.enter_context(tc.tile_pool(name="qstage", bufs=8))
            ps_zg = ctx.enter_context(tc.tile_pool(name="zg", bufs=2, space="PSUM"))
            ps_zc = ctx.enter_context(tc.tile_pool(name="zc", bufs=2, space="PSUM"))
            ps_bq = ctx.enter_context(tc.tile_pool(name="bq", bufs=2, space="PSUM"))

            # ---- constants (preloaded once) ----
            w1_sb = consts.tile([D + H, 2 * H], f32, tag="w1")
            nc.sync.dma_start(out=R(w1_sb[:]), in_=R(W1[:]))
            adjb_sb = consts.tile([1, 2 * H], f32, tag="adjb")
            nc.sync.dma_start(out=R(adjb_sb[:]), in_=R(ADJB[:]))
            # w2x lives at partitions 64:128 (must match rhs base = 64, the x rows)
            w2x_sb = consts.tile([D + H, H], f32, tag="w2x")
            nc.sync.dma_start(out=R(w2x_sb[64:128, :]), in_=R(W2X[:]))
            w2he_sb = consts.tile([H + 1, H], f32, tag="w2he")
            nc.sync.dma_start(out=R(w2he_sb[:]), in_=R(W2HE[:]))
            ones_sb = consts.tile([1, BL], f32, tag="ones")
            nc.sync.dma_start(out=R(ones_sb[:]), in_=R(ONES[0:1, :]))

            # ---- state tiles ----
            # xh[j]: rows 0:64 = h~ (base 0), rows 64:128 = x_t (base 64)
            NXH = 4
            xh = [state.tile([128, BL], f32, tag=f"xh{j}", name=f"xh{j}")
                  for j in range(NXH)]
            # h~_0 = h_0 + 1 = 1
            nc.sync.dma_start(out=R(xh[0][0:64, :]), in_=R(ONES[:]))
            # rh_ext[k]: rows 0:64 = r*h (written per step), row 64 = ones (bias row)
            NRH = 2
            rhe = [state.tile([H + 1, BL], f32, tag=f"rhe{k}", name=f"rhe{k}")
                   for k in range(NRH)]
            for k in range(NRH):
                nc.sync.dma_start(out=R(rhe[k][64:65, :]), in_=R(ONES[0:1, :]))

            # ---- the recurrence ----
            for t in range(T):
                cur = xh[t % NXH]
                nxt = xh[(t + 1) % NXH]
                rh_t = rhe[t % NRH]

                # x_t -> rows 64:128 of the current xh tile (prefetchable ~NXH steps)
                nc.sync.dma_start(out=R(cur[64:128, :]), in_=R(X[t]))
                # q_t staging (tiny; deep prefetch via pool bufs)
                qs = qpool.tile([1, BL], f32, tag="qs")
                nc.gpsimd.dma_start(out=R(qs[:]), in_=R(Qd[t : t + 1, :]))

                # zg = adjb ⊕ W1'.T @ [h~; x]
                zg = ps_zg.tile([2 * H, BL], f32, tag="zg")
                nc.tensor.matmul(zg[:], lhsT=R(adjb_sb[:]), rhs=R(ones_sb[:]),
                                 start=True, stop=False)
                nc.tensor.matmul(zg[:], lhsT=R(w1_sb[:]), rhs=R(cur[:]),
                                 start=False, stop=True)

                # bq = broadcast of q_t over 64 partitions (PSUM)
                bq = ps_bq.tile([H, BL], f32, tag="bq")
                nc.tensor.matmul(bq[:], lhsT=R(ones_sb[:, 0:H]), rhs=R(qs[:]),
                                 start=True, stop=True)

                # gates = sigmoid(zg): r = G[0:64] (b0), u = G[64:128] (b64)
                G = gpool.tile([2 * H, BL], f32, tag="G")
                nc.scalar.activation(G[:], zg[:], AF.Sigmoid)

                # u_hat = bq * u   (PSUM b0 x SBUF b64 -> SBUF b0)
                uh = tpool.tile([H, BL], f32, tag="uh")
                nc.vector.tensor_tensor(uh[:], bq[:], G[64:128, :], op=ALU.mult)

                # rh = (h~ - 1) * r
                nc.vector.scalar_tensor_tensor(R(rh_t[0:64, :]), cur[0:64, :], 1.0,
                                               G[0:64, :], op0=ALU.subtract, op1=ALU.mult)

                # zc = 2*(Wc.T [x, r*h] + cb) via pre-scaled weights + ones row
                zc = ps_zc.tile([H, BL], f32, tag="zc")
                nc.tensor.matmul(zc[:], lhsT=R(w2he_sb[:]), rhs=R(rh_t[:]),
                                 start=True, stop=False)
                nc.tensor.matmul(zc[:], lhsT=R(w2x_sb[64:128, :]), rhs=R(cur[64:128, :]),
                                 start=False, stop=True)

                # s = sigmoid(zc);  c = 2s - 1
                gc = gpool.tile([H, BL], f32, tag="gc")
                nc.scalar.activation(gc[:], zc[:], AF.Sigmoid)

                # d' = 2s - h~  (= c - h)
                dp = tpool.tile([H, BL], f32, tag="dp")
                nc.vector.scalar_tensor_tensor(dp[:], gc[:], 2.0, cur[0:64, :],
                                               op0=ALU.mult, op1=ALU.subtract)
                # e' = u_hat * d'
                ep = tpool.tile([H, BL], f32, tag="ep")
                nc.vector.tensor_tensor(ep[:], uh[:], dp[:], op=ALU.mult)
                # h~' = 2s - e'
                nc.vector.scalar_tensor_tensor(R(nxt[0:64, :]), gc[:], 2.0, ep[:],
                                               op0=ALU.mult, op1=ALU.subtract)

                # out_t = h~' (host subtracts 1 and masks)
                nc.gpsimd.dma_start(out=OUT[t], in_=nxt[0:64, :])

    nc.compile()
    _CACHE["nc"] = nc
    return nc


def _prep_shared(gate_kernel, gate_bias, cand_kernel, cand_bias):
    gk = np.asarray(gate_kernel, np.float32)
    gb = np.asarray(gate_bias, np.float32)
    ck = np.asarray(cand_kernel, np.float32)
    cb = np.asarray(cand_bias, np.float32)
    # xh rows: 0:64 = h -> gate_kernel h-rows; 64:128 = x -> x-rows
    w1p = np.concatenate([gk[D:, :], gk[:D, :]], axis=0)            # [128,128]
    adjb = (gb - gk[D:, :].sum(axis=0)).reshape(1, 2 * H)           # h~ = h+1 fix
    w2x = 2.0 * ck[:D, :]                                            # [64,64]
    w2he = np.concatenate([2.0 * ck[D:, :], 2.0 * cb[None, :]], 0)   # [65,64]
    ones = np.ones((H, BL), np.float32)
    return (np.ascontiguousarray(w1p), np.ascontiguousarray(adjb),
            np.ascontiguousarray(w2x), np.ascontiguousarray(w2he), ones)


def _run(inputs, trace=False):
    from concourse.bass_utils import run_bass_kernel_spmd

    nc = _build()
    rnn_input = np.asarray(inputs["rnn_input"], np.float32)
    seq_len = np.asarray(inputs["sequence_length"], np.int32)
    att = np.asarray(inputs["att_score"], np.float32)
    w1p, adjb, w2x, w2he, ones = _prep_shared(
        inputs["gate_kernel"], inputs["gate_bias"],
        inputs["cand_kernel"], inputs["cand_bias"])

    in_maps = []
    for i in range(NCORES):
        b0, b1 = i * BL, (i + 1) * BL
        xi = np.ascontiguousarray(rnn_input[b0:b1].transpose(1, 2, 0))  # [T,D,BL]
        qi = np.ascontiguousarray(1.0 - att[b0:b1, :, 0].T)             # [T,BL]
        in_maps.append({"x": xi, "q": qi, "w1": w1p, "adjb": adjb,
                        "w2x": w2x, "w2he": w2he, "ones": ones})

    res = run_bass_kernel_spmd(nc, in_maps, core_ids=list(range(NCORES)), trace=trace)

    out = np.empty((B, T, H), np.float32)
    t_idx = np.arange(T, dtype=np.int32)
    for i in range(NCORES):
        b0, b1 = i * BL, (i + 1) * BL
        y = res.results[i]["out"]                      # [T,H,BL] = h~'
        hi = y.transpose(2, 0, 1) - 1.0                # [BL,T,H]
        mask = (t_idx[None, :] < seq_len[b0:b1, None]).astype(np.float32)
        out[b0:b1] = hi * mask[:, :, None]
    return out, res


def kernel(**inputs):
    out, _ = _run(inputs)
    return out


# revision 13
# speedup vs baseline: 7544.4890x; 7544.4890x over previous
"""AUGRU (attention-modulated GRU) Trainium2 kernel.

Problem: B=2048, T=200, D=H=64.  Data-parallel over 8 NeuronCores
(256 batch rows per core); the T=200 recurrence runs on-device.

Math per step (reference):
    gates = sigmoid([x, h] @ Wg + bg);  r, u = split(gates)
    c     = tanh([x, r*h] @ Wc + bc)
    u_hat = (1 - att) * u
    h'    = u_hat * h + (1 - u_hat) * c
    out_t = valid ? h' : 0;  h = valid ? h' : h      (valid = t < seq_len)

Device-side reformulation (feature-major layout [feat, batch]):
  * The valid-mask is NOT applied on device: invalid steps only corrupt
    outputs at t >= seq_len, which the host zeroes afterwards.
  * State kept as h~ = h + 1 so that affine corrections fold into
    scalar_tensor_tensor ops; weight/bias corrections are folded on host.
  * tanh(z) = 2*sigmoid(2z) - 1: candidate weights pre-scaled by 2 on host
    so only Sigmoid is ever evaluated (one ACT table set).
  * s  = sigmoid(2*zc);  c - h = 2s - h~;  h~' = 2s - u_hat*(2s - h~).
  * (1-att) is broadcast across the 64 feature partitions with a K=1 matmul.
Matmuls run in float32r (full-rate fp32 PE mode, ~1e-5 rel precision).
"""

import numpy as np

B, T, D, H = 2048, 200, 64, 64
NCORES = 8
BL = B // NCORES  # 256

# "f32": f32 storage everywhere, float32r matmuls (safest precision).
# "bf16": bf16 storage for x/h/gates/weights -> 2x DVE modes, halved DMA.
EW_DT = "f32"

_CACHE = {}


def _build():
    """Build + compile the per-core Bass module (same SPMD program on all cores)."""
    if "nc" in _CACHE:
        return _CACHE["nc"]

    from contextlib import ExitStack
    import concourse.tile as tile
    from concourse import bacc, mybir

    f32 = mybir.dt.float32
    f32r = mybir.dt.float32r
    ALU = mybir.AluOpType
    AF = mybir.ActivationFunctionType

    bf16 = mybir.dt.bfloat16
    ew = bf16 if EW_DT == "bf16" else f32

    nc = bacc.Bacc("TRN2", target_bir_lowering=False, debug=False,
                   enable_asserts=False, num_devices=NCORES)

    X = nc.dram_tensor("x", [T, D, BL], ew, kind="ExternalInput").ap()
    Qd = nc.dram_tensor("q", [T, BL], ew, kind="ExternalInput").ap()
    W1 = nc.dram_tensor("w1", [D + H, 2 * H], ew, kind="ExternalInput").ap()
    ADJB = nc.dram_tensor("adjb", [1, 2 * H], ew, kind="ExternalInput").ap()
    W2X = nc.dram_tensor("w2x", [D, H], ew, kind="ExternalInput").ap()
    W2HE = nc.dram_tensor("w2he", [H + 1, H], ew, kind="ExternalInput").ap()
    ONES = nc.dram_tensor("ones", [H, BL], ew, kind="ExternalInput").ap()
    OUT = nc.dram_tensor("out", [T, H, BL], ew, kind="ExternalOutput").ap()

    def R(ap):
        # f32 mode: PE consumes float32r (requires declared rounding);
        # bf16 mode: PE consumes bf16 directly, no bitcast needed.
        return ap.bitcast(f32r) if ew is not bf16 else ap

    with tile.TileContext(nc) as tc:
        with ExitStack() as ctx:
            consts = ctx.enter_context(tc.tile_pool(name="consts", bufs=1))
            state = ctx.enter_context(tc.tile_pool(name="state", bufs=1))
            gpool = ctx.enter_context(tc.tile_pool(name="gates", bufs=3))
            tpool = ctx.enter_context(tc.tile_pool(name="tmp", bufs=3))
            qpool = ctx.enter_context(tc.tile_pool(name="qstage", bufs=8))
            ps_zg = ctx.enter_context(tc.tile_pool(name="zg", bufs=2, space="PSUM"))
            ps_zc = ctx.enter_context(tc.tile_pool(name="zc", bufs=2, space="PSUM"))
            ps_bq = ctx.enter_context(tc.tile_pool(name="bq", bufs=2, space="PSUM"))

            # ---- constants (preloaded once) ----
            w1_sb = consts.tile([D + H, 2 * H], ew, tag="w1")
            nc.sync.dma_start(out=R(w1_sb[:]), in_=R(W1[:]))
            adjb_sb = consts.tile([1, 2 * H], ew, tag="adjb")
            nc.sync.dma_start(out=R(adjb_sb[:]), in_=R(ADJB[:]))
            # w2x lives at partitions 64:128 (must match rhs base = 64, the x rows)
            w2x_sb = consts.tile([D + H, H], ew, tag="w2x")
            nc.sync.dma_start(out=R(w2x_sb[64:128, :]), in_=R(W2X[:]))
            w2he_sb = consts.tile([H + 1, H], ew, tag="w2he")
            nc.sync.dma_start(out=R(w2he_sb[:]), in_=R(W2HE[:]))
            ones_sb = consts.tile([1, BL], ew, tag="ones")
            nc.sync.dma_start(out=R(ones_sb[:]), in_=R(ONES[0:1, :]))

            # ---- state tiles ----
            # xh[j]: rows 0:64 = h~ (base 0), rows 64:128 = x_t (base 64)
            NXH = 4
            xh = [state.tile([128, BL], ew, tag=f"xh{j}", name=f"xh{j}")
                  for j in range(NXH)]
            # h~_0 = h_0 + 1 = 1
            nc.sync.dma_start(out=R(xh[0][0:64, :]), in_=R(ONES[:]))
            # rh_ext[k]: rows 0:64 = r*h (written per step), row 64 = ones (bias row)
            NRH = 2
            rhe = [state.tile([H + 1, BL], ew, tag=f"rhe{k}", name=f"rhe{k}")
                   for k in range(NRH)]
            for k in range(NRH):
                nc.sync.dma_start(out=R(rhe[k][64:65, :]), in_=R(ONES[0:1, :]))

            # ---- the recurrence ----
            for t in range(T):
                cur = xh[t % NXH]
                nxt = xh[(t + 1) % NXH]
                rh_t = rhe[t % NRH]

                # x_t -> rows 64:128 of the current xh tile (prefetchable ~NXH steps)
                nc.sync.dma_start(out=R(cur[64:128, :]), in_=R(X[t]))
                # q_t staging (tiny; deep prefetch via pool bufs)
                qs = qpool.tile([1, BL], ew, tag="qs")
                nc.sync.dma_start(out=R(qs[:]), in_=R(Qd[t : t + 1, :]))

                # zg = adjb ⊕ W1'.T @ [h~; x]
                zg = ps_zg.tile([2 * H, BL], f32, tag="zg")
                nc.tensor.matmul(zg[:], lhsT=R(adjb_sb[:]), rhs=R(ones_sb[:]),
                                 start=True, stop=False)
                nc.tensor.matmul(zg[:], lhsT=R(w1_sb[:]), rhs=R(cur[:]),
                                 start=False, stop=True)

                # bq = broadcast of q_t over 64 partitions (PSUM)
                bq = ps_bq.tile([H, BL], f32, tag="bq")
                nc.tensor.matmul(bq[:], lhsT=R(ones_sb[:, 0:H]), rhs=R(qs[:]),
                                 start=True, stop=True)

                # gates = sigmoid(zg): r = G[0:64] (b0), u = G[64:128] (b64)
                G = gpool.tile([2 * H, BL], ew, tag="G")
                nc.scalar.activation(G[:], zg[:], AF.Sigmoid)

                # rh = (h~ - 1) * r   [chain-critical: keep first in DVE FIFO]
                nc.vector.scalar_tensor_tensor(R(rh_t[0:64, :]), cur[0:64, :], 1.0,
                                               G[0:64, :], op0=ALU.subtract, op1=ALU.mult)

                # u_hat = bq * u   (PSUM b0 x SBUF b64 -> SBUF b0)  [off-chain]
                uh = tpool.tile([H, BL], ew, tag="uh")
                nc.vector.tensor_tensor(uh[:], bq[:], G[64:128, :], op=ALU.mult)
                # A2 = 2 - 2*u_hat  (off-chain, DVE)
                a2 = tpool.tile([H, BL], ew, tag="a2")
                nc.vector.tensor_scalar(a2[:], uh[:], -2.0, 2.0,
                                        op0=ALU.mult, op1=ALU.add)
                # Bt = u_hat * h~  (off-chain, GPSIMD)
                bt = tpool.tile([H, BL], ew, tag="bt")
                nc.gpsimd.tensor_tensor(bt[:], uh[:], cur[0:64, :], op=ALU.mult)

                # zc = 2*(Wc.T [x, r*h] + cb) via pre-scaled weights + ones row.
                # x-part first (its data is prefetched) so only the rh matmul
                # sits on the recurrence-critical path.
                zc = ps_zc.tile([H, BL], f32, tag="zc")
                nc.tensor.matmul(zc[:], lhsT=R(w2x_sb[64:128, :]), rhs=R(cur[64:128, :]),
                                 start=True, stop=False)
                nc.tensor.matmul(zc[:], lhsT=R(w2he_sb[:]), rhs=R(rh_t[:]),
                                 start=False, stop=True)

                # s = sigmoid(zc);  c = 2s - 1
                gc = gpool.tile([H, BL], ew, tag="gc")
                nc.scalar.activation(gc[:], zc[:], AF.Sigmoid)

                # h~' = A2*s + Bt = 2s(1-u_hat) + u_hat*h~   [2-hop chain tail]
                m = tpool.tile([H, BL], ew, tag="m")
                nc.vector.tensor_tensor(m[:], gc[:], a2[:], op=ALU.mult)
                nc.vector.tensor_tensor(R(nxt[0:64, :]), m[:], bt[:], op=ALU.add)

                # out_t = h~' (host subtracts 1 and masks)
                nc.sync.dma_start(out=OUT[t], in_=nxt[0:64, :])

    nc.compile()
    _CACHE["nc"] = nc
    return nc


def _prep_shared(gate_kernel, gate_bias, cand_kernel, cand_bias):
    gk = np.asarray(gate_kernel, np.float32)
    gb = np.asarray(gate_bias, np.float32)
    ck = np.asarray(cand_kernel, np.float32)
    cb = np.asarray(cand_bias, np.float32)
    # xh rows: 0:64 = h -> gate_kernel h-rows; 64:128 = x -> x-rows
    w1p = np.concatenate([gk[D:, :], gk[:D, :]], axis=0)            # [128,128]
    adjb = (gb - gk[D:, :].sum(axis=0)).reshape(1, 2 * H)           # h~ = h+1 fix
    w2x = 2.0 * ck[:D, :]                                            # [64,64]
    w2he = np.concatenate([2.0 * ck[D:, :], 2.0 * cb[None, :]], 0)   # [65,64]
    ones = np.ones((H, BL), np.float32)
    return (np.ascontiguousarray(w1p), np.ascontiguousarray(adjb),
            np.ascontiguousarray(w2x), np.ascontiguousarray(w2he), ones)


def _np_dt():
    if EW_DT == "bf16":
        import ml_dtypes
        return np.dtype(ml_dtypes.bfloat16)
    return np.dtype(np.float32)


def _run(inputs, trace=False):
    from concourse.bass_utils import run_bass_kernel_spmd

    nc = _build()
    ndt = _np_dt()
    rnn_input = np.asarray(inputs["rnn_input"], np.float32)
    seq_len = np.asarray(inputs["sequence_length"], np.int32)
    att = np.asarray(inputs["att_score"], np.float32)
    w1p, adjb, w2x, w2he, ones = _prep_shared(
        inputs["gate_kernel"], inputs["gate_bias"],
        inputs["cand_kernel"], inputs["cand_bias"])

    in_maps = []
    for i in range(NCORES):
        b0, b1 = i * BL, (i + 1) * BL
        xi = np.ascontiguousarray(rnn_input[b0:b1].transpose(1, 2, 0)).astype(ndt)
        qi = np.ascontiguousarray(1.0 - att[b0:b1, :, 0].T).astype(ndt)
        in_maps.append({"x": xi, "q": qi, "w1": w1p.astype(ndt), "adjb": adjb.astype(ndt),
                        "w2x": w2x.astype(ndt), "w2he": w2he.astype(ndt),
                        "ones": ones.astype(ndt)})

    res = run_bass_kernel_spmd(nc, in_maps, core_ids=list(range(NCORES)), trace=trace)

    out = np.empty((B, T, H), np.float32)
    t_idx = np.arange(T, dtype=np.int32)
    for i in range(NCORES):
        b0, b1 = i * BL, (i + 1) * BL
        y = np.asarray(res.results[i]["out"], np.float32)   # [T,H,BL] = h~'
        hi = y.transpose(2, 0, 1) - 1.0                # [BL,T,H]
        mask = (t_idx[None, :] < seq_len[b0:b1, None]).astype(np.float32)
        out[b0:b1] = hi * mask[:, :, None]
    return out, res


def kernel(**inputs):
    out, _ = _run(inputs)
    return out


# revision 27
# speedup vs baseline: 8748.3259x; 1.1596x over previous
"""AUGRU (attention-modulated GRU) Trainium2 kernel.

Problem: B=2048, T=200, D=H=64.  Data-parallel over 8 NeuronCores
(256 batch rows per core); the T=200 recurrence runs on-device.

Math per step (reference):
    gates = sigmoid([x, h] @ Wg + bg);  r, u = split(gates)
    c     = tanh([x, r*h] @ Wc + bc)
    u_hat = (1 - att) * u
    h'    = u_hat * h + (1 - u_hat) * c
    out_t = valid ? h' : 0;  h = valid ? h' : h      (valid = t < seq_len)

Device-side reformulation (feature-major layout [feat, batch]):
  * The valid-mask is NOT applied on device: invalid steps only corrupt
    outputs at t >= seq_len, which the host zeroes afterwards.
  * State kept as h~ = h + 1 so that affine corrections fold into
    scalar_tensor_tensor ops; weight/bias corrections are folded on host.
  * tanh(z) = 2*sigmoid(2z) - 1: candidate weights pre-scaled by 2 on host
    so only Sigmoid is ever evaluated (one ACT table set).
  * s  = sigmoid(2*zc);  c - h = 2s - h~;  h~' = 2s - u_hat*(2s - h~),
    computed as h~' = m + B with m = 2s(1-u_hat) (chain-late) and
    B = u_hat*h~ (early).  The next step's gate matmul consumes m and B
    via two separate accumulating matmuls, so materializing h~' leaves
    the recurrence-critical path entirely.
  * (1-att) is broadcast across the 64 feature partitions with a K=1 matmul.
Matmuls run in float32r (full-rate fp32 PE mode, ~1e-5 rel precision).
NOTE: PSUM accumulation groups of 4+ f32r matmuls crash TRN2 at runtime
(clean compile, NRT INTERNAL error on execute); 3 per group is the max
used here (bias folded into the x-matmul via a ones-row, K=65).
"""

import numpy as np

B, T, D, H = 2048, 200, 64, 64
NCORES = 8
BL = B // NCORES  # 256

# "f32": f32 storage everywhere, float32r matmuls (safest precision).
# "bf16": bf16 storage for x/h/gates/weights -> 2x DVE modes, halved DMA.
EW_DT = "f32"

_CACHE = {}


def _build():
    """Build + compile the per-core Bass module (same SPMD program on all cores)."""
    if "nc" in _CACHE:
        return _CACHE["nc"]

    from contextlib import ExitStack
    import concourse.tile as tile
    from concourse import bacc, mybir

    f32 = mybir.dt.float32
    f32r = mybir.dt.float32r
    ALU = mybir.AluOpType
    AF = mybir.ActivationFunctionType

    bf16 = mybir.dt.bfloat16
    ew = bf16 if EW_DT == "bf16" else f32

    nc = bacc.Bacc("TRN2", target_bir_lowering=False, debug=False,
                   enable_asserts=False, num_devices=NCORES)

    X = nc.dram_tensor("x", [T, D, BL], ew, kind="ExternalInput").ap()
    Qd = nc.dram_tensor("q", [T, BL], ew, kind="ExternalInput").ap()
    W1H = nc.dram_tensor("w1h", [H, 2 * H], ew, kind="ExternalInput").ap()
    W1XB = nc.dram_tensor("w1xb", [D + 1, 2 * H], ew, kind="ExternalInput").ap()
    W2X = nc.dram_tensor("w2x", [D, H], ew, kind="ExternalInput").ap()
    W2HE = nc.dram_tensor("w2he", [H + 1, H], ew, kind="ExternalInput").ap()
    ONES = nc.dram_tensor("ones", [H, BL], ew, kind="ExternalInput").ap()
    OUT = nc.dram_tensor("out", [T, H, BL], ew, kind="ExternalOutput").ap()

    def R(ap):
        # f32 mode: PE consumes float32r (requires declared rounding);
        # bf16 mode: PE consumes bf16 directly, no bitcast needed.
        return ap.bitcast(f32r) if ew is not bf16 else ap

    with tile.TileContext(nc) as tc:
        with ExitStack() as ctx:
            consts = ctx.enter_context(tc.tile_pool(name="consts", bufs=1))
            state = ctx.enter_context(tc.tile_pool(name="state", bufs=1))
            gpool = ctx.enter_context(tc.tile_pool(name="gates", bufs=3))
            tpool = ctx.enter_context(tc.tile_pool(name="tmp", bufs=3))
            qpool = ctx.enter_context(tc.tile_pool(name="qstage", bufs=8))
            ps_zg = ctx.enter_context(tc.tile_pool(name="zg", bufs=2, space="PSUM"))
            ps_zc = ctx.enter_context(tc.tile_pool(name="zc", bufs=2, space="PSUM"))
            ps_bq = ctx.enter_context(tc.tile_pool(name="bq", bufs=2, space="PSUM"))

            # ---- constants (preloaded once) ----
            w1h_sb = consts.tile([H, 2 * H], ew, tag="w1h")
            nc.sync.dma_start(out=R(w1h_sb[:]), in_=R(W1H[:]))
            w1xb_sb = consts.tile([D + 1, 2 * H], ew, tag="w1xb")
            nc.sync.dma_start(out=R(w1xb_sb[:]), in_=R(W1XB[:]))
            w2x_sb = consts.tile([D, H], ew, tag="w2x")
            nc.sync.dma_start(out=R(w2x_sb[:]), in_=R(W2X[:]))
            w2he_sb = consts.tile([H + 1, H], ew, tag="w2he")
            nc.sync.dma_start(out=R(w2he_sb[:]), in_=R(W2HE[:]))
            ones_sb = consts.tile([1, BL], ew, tag="ones")
            nc.sync.dma_start(out=R(ones_sb[:]), in_=R(ONES[0:1, :]))

            # ---- state tiles ----
            NXH = 4
            ht = [state.tile([H, BL], ew, tag=f"ht{j}", name=f"ht{j}")
                  for j in range(NXH)]
            xt = [state.tile([D + 1, BL], ew, tag=f"xt{j}", name=f"xt{j}")
                  for j in range(NXH)]
            # h~_0 = h_0 + 1 = 1;  xt ones-row (bias row for W1XB) in each buffer
            nc.sync.dma_start(out=R(ht[0][:]), in_=R(ONES[:]))
            for j in range(NXH):
                nc.sync.dma_start(out=R(xt[j][64:65, :]), in_=R(ONES[0:1, :]))
            # m/Bt: persistent rotating tiles (they feed next step's matmuls)
            mt = [state.tile([H, BL], ew, tag=f"mt{k}", name=f"mt{k}")
                  for k in range(2)]
            btt = [state.tile([H, BL], ew, tag=f"btt{k}", name=f"btt{k}")
                   for k in range(2)]
            # rh_ext[k]: rows 0:64 = r*h (written per step), row 64 = ones (bias row)
            NRH = 2
            rhe = [state.tile([H + 1, BL], ew, tag=f"rhe{k}", name=f"rhe{k}")
                   for k in range(NRH)]
            for k in range(NRH):
                nc.sync.dma_start(out=R(rhe[k][64:65, :]), in_=R(ONES[0:1, :]))

            # ---- the recurrence ----
            for t in range(T):
                hcur = ht[t % NXH]
                hnxt = ht[(t + 1) % NXH]
                xcur = xt[t % NXH]
                rh_t = rhe[t % NRH]

                # x_t -> rows 0:64 of the x tile (prefetchable ~NXH steps)
                nc.sync.dma_start(out=R(xcur[0:64, :]), in_=R(X[t]))
                # q_t staging (tiny; deep prefetch via pool bufs)
                qs = qpool.tile([1, BL], ew, tag="qs")
                nc.sync.dma_start(out=R(qs[:]), in_=R(Qd[t : t + 1, :]))

                # zg = (W1x|adjb).T @ (x|1)  +  W1h.T @ B(t-1)  +  W1h.T @ m(t-1)
                # (h~ = m + B; the B matmul runs early, only m's is chain-late;
                #  max 3 matmuls per accumulation group)
                zg = ps_zg.tile([2 * H, BL], f32, tag="zg")
                nc.tensor.matmul(zg[:], lhsT=R(w1xb_sb[:]), rhs=R(xcur[:]),
                                 start=True, stop=False)
                if t == 0:
                    nc.tensor.matmul(zg[:], lhsT=R(w1h_sb[:]), rhs=R(hcur[:]),
                                     start=False, stop=True)
                else:
                    nc.tensor.matmul(zg[:], lhsT=R(w1h_sb[:]), rhs=R(btt[(t - 1) % 2][:]),
                                     start=False, stop=False)
                    nc.tensor.matmul(zg[:], lhsT=R(w1h_sb[:]), rhs=R(mt[(t - 1) % 2][:]),
                                     start=False, stop=True)

                # bq = broadcast of q_t over 64 partitions (PSUM)
                bq = ps_bq.tile([H, BL], f32, tag="bq")
                nc.tensor.matmul(bq[:], lhsT=R(ones_sb[:, 0:H]), rhs=R(qs[:]),
                                 start=True, stop=True)

                # gates = sigmoid(zg): r = G[0:64] (b0), u = G[64:128] (b64)
                G = gpool.tile([2 * H, BL], ew, tag="G")
                nc.scalar.activation(G[:], zg[:], AF.Sigmoid)

                # rh = (h~ - 1) * r   [chain-critical: keep first in DVE FIFO]
                nc.vector.scalar_tensor_tensor(R(rh_t[0:64, :]), hcur[:], 1.0,
                                               G[0:64, :], op0=ALU.subtract, op1=ALU.mult)

                # u_hat = bq * u   (PSUM b0 x SBUF b64 -> SBUF b0)  [off-chain]
                uh = tpool.tile([H, BL], ew, tag="uh")
                nc.vector.tensor_tensor(uh[:], bq[:], G[64:128, :], op=ALU.mult)
                # A2 = 2 - 2*u_hat  (off-chain, DVE)
                a2 = tpool.tile([H, BL], ew, tag="a2")
                nc.vector.tensor_scalar(a2[:], uh[:], -2.0, 2.0,
                                        op0=ALU.mult, op1=ALU.add)
                # Bt = u_hat * h~  (off-chain, DVE; f32r out feeds next MM)
                bt = btt[t % 2]
                nc.vector.tensor_tensor(R(bt[:]), uh[:], hcur[:], op=ALU.mult)

                # zc = 2*(Wc.T [x, r*h] + cb) via pre-scaled weights + ones row.
                # x-part first (its data is prefetched) so only the rh matmul
                # sits on the recurrence-critical path.
                zc = ps_zc.tile([H, BL], f32, tag="zc")
                nc.tensor.matmul(zc[:], lhsT=R(w2x_sb[:]), rhs=R(xcur[0:64, :]),
                                 start=True, stop=False)
                nc.tensor.matmul(zc[:], lhsT=R(w2he_sb[:]), rhs=R(rh_t[:]),
                                 start=False, stop=True)

                # s = sigmoid(zc);  c = 2s - 1
                gc = gpool.tile([H, BL], ew, tag="gc")
                nc.scalar.activation(gc[:], zc[:], AF.Sigmoid)

                # m = A2*s  [only chain-tail op; f32r out feeds next MM1m]
                m = mt[t % 2]
                nc.vector.tensor_tensor(R(m[:]), gc[:], a2[:], op=ALU.mult)
                # h~' = m + Bt  (off-chain: only rh/Bt(t+1) and OUT read it)
                nc.vector.tensor_tensor(R(hnxt[:]), m[:], bt[:], op=ALU.add)

                # out_t = h~' (host subtracts 1 and masks)
                nc.sync.dma_start(out=OUT[t], in_=hnxt[:])

    nc.compile()
    _CACHE["nc"] = nc
    return nc


def _prep_shared(gate_kernel, gate_bias, cand_kernel, cand_bias):
    gk = np.asarray(gate_kernel, np.float32)
    gb = np.asarray(gate_bias, np.float32)
    ck = np.asarray(cand_kernel, np.float32)
    cb = np.asarray(cand_bias, np.float32)
    # xh rows: 0:64 = h -> gate_kernel h-rows; 64:128 = x -> x-rows
    w1p = np.concatenate([gk[D:, :], gk[:D, :]], axis=0)            # [128,128]
    adjb = (gb - gk[D:, :].sum(axis=0)).reshape(1, 2 * H)           # h~ = h+1 fix
    w2x = 2.0 * ck[:D, :]                                            # [64,64]
    w2he = np.concatenate([2.0 * ck[D:, :], 2.0 * cb[None, :]], 0)   # [65,64]
    ones = np.ones((H, BL), np.float32)
    return (np.ascontiguousarray(w1p), np.ascontiguousarray(adjb),
            np.ascontiguousarray(w2x), np.ascontiguousarray(w2he), ones)


def _np_dt():
    if EW_DT == "bf16":
        import ml_dtypes
        return np.dtype(ml_dtypes.bfloat16)
    return np.dtype(np.float32)


def _run(inputs, trace=False):
    from concourse.bass_utils import run_bass_kernel_spmd

    nc = _build()
    ndt = _np_dt()
    rnn_input = np.asarray(inputs["rnn_input"], np.float32)
    seq_len = np.asarray(inputs["sequence_length"], np.int32)
    att = np.asarray(inputs["att_score"], np.float32)
    w1p, adjb, w2x, w2he, ones = _prep_shared(
        inputs["gate_kernel"], inputs["gate_bias"],
        inputs["cand_kernel"], inputs["cand_bias"])

    in_maps = []
    for i in range(NCORES):
        b0, b1 = i * BL, (i + 1) * BL
        xi = np.ascontiguousarray(rnn_input[b0:b1].transpose(1, 2, 0)).astype(ndt)
        qi = np.ascontiguousarray(1.0 - att[b0:b1, :, 0].T).astype(ndt)
        w1h = w1p[0:H, :]                               # h-rows
        w1xb = np.concatenate([w1p[H:, :], adjb], 0)    # x-rows + bias row
        in_maps.append({"x": xi, "q": qi, "w1h": w1h.astype(ndt),
                        "w1xb": np.ascontiguousarray(w1xb).astype(ndt),
                        "w2x": w2x.astype(ndt), "w2he": w2he.astype(ndt),
                        "ones": ones.astype(ndt)})

    res = run_bass_kernel_spmd(nc, in_maps, core_ids=list(range(NCORES)), trace=trace)

    out = np.empty((B, T, H), np.float32)
    t_idx = np.arange(T, dtype=np.int32)
    for i in range(NCORES):
        b0, b1 = i * BL, (i + 1) * BL
        y = np.asarray(res.results[i]["out"], np.float32)   # [T,H,BL] = h~'
        hi = y.transpose(2, 0, 1) - 1.0                # [BL,T,H]
        mask = (t_idx[None, :] < seq_len[b0:b1, None]).astype(np.float32)
        out[b0:b1] = hi * mask[:, :, None]
    return out, res


def kernel(**inputs):
    out, _ = _run(inputs)
    return out
